# revision 41
# baseline (speedup 1.0000x reference)
"""DYSPN attention-conv kernel for Trainium2 (8 NeuronCores, batch-parallel).

Math (unfold/fold pair collapses algebraically; per image, tap k=(i,j) != center,
ring r = INDEX[i,j], dy = 3-i, dx = 3-j):
    z_k[y,x] = att_r[y,x] * aff_k[y,x]
    U[y,x]   = sum_k z_k[y,x]                       (S_ppt - att3)
    A[y,x]   = sum_k |z_k[y,x]|                     (S_prime - att3; att >= 0)
    T[y,x]   = sum_k z_k[y+dy, x+dx]  (in-image)    (fold7(z))
    out = r * ((T+att3)*cs - (U+att3)*co) + co,  r = 1/(A+att3+eps)

Design: 2 images/core; one bf16 z tile per image holding all 48 taps and BOTH
128-row blocks ([128, 48, 2, 264]) so cross-image double buffering fits in
SBUF and the T halo is chunk-local. fp32 affinity chunks DMA into a rotating
fp32 staging pool (sync/HWDGE ring, nothing else on it), DVE multiplies by
the ring-broadcast attention -> bf16 z. All reductions are PE bf16 matmuls
with 512-wide rhs covering both blocks: U/A via the identity band, T via
dy-shifted band windows plus per-tap halo matmuls. |z| on ACT (4-tap ops).
Guard zeroing via one-time Pool memsets. Other inputs ride the scalar HWDGE
+ gpsimd SWDGE rings; image 1's inputs are emitted behind image 0's early
ACT work so they don't steal bandwidth from chunk 0. Epilogue on DVE with
reciprocal_approx_fast; out store on the SWDGE ring.
"""
import sys

sys.path.insert(0, "/opt/trn_rl_repo")

import numpy as np
from ml_dtypes import bfloat16

import concourse.bass as bass  # noqa: F401  (registers engines)
import concourse.tile as tile
from concourse import bacc, mybir
from concourse.bass_utils import run_bass_kernel_spmd

FP32 = mybir.dt.float32
BF16 = mybir.dt.bfloat16

N_CORES = 8
B_FULL = 16
B_CORE = B_FULL // N_CORES  # 2 images per core
H = W = 256
K = 7
NT = 48                   # 49 taps minus center
GW = 4                    # zero guard columns each side of a z tap plane
WG = W + 2 * GW
BANDW = 390               # band[p, q] = 1 iff q == p + C0
C0 = 131
EPS = 1e-6
# tap chunks (t-order); edges never cross t=24 (the skipped center plane);
# graded small at both ends for a fast start and a short tail
CHUNKS = [(0, 4), (4, 8), (8, 16), (16, 24),
          (24, 32), (32, 40), (40, 44), (44, 46), (46, 48)]
STG_TAPS = 8              # staging tile capacity (max chunk size)

_INDEX = np.array([0, 0, 0, 0, 0, 0, 0,
                   0, 1, 1, 1, 1, 1, 0,
                   0, 1, 2, 2, 2, 1, 0,
                   0, 1, 2, 3, 2, 1, 0,
                   0, 1, 2, 2, 2, 1, 0,
                   0, 1, 1, 1, 1, 1, 0,
                   0, 0, 0, 0, 0, 0, 0], dtype=np.int64).reshape(7, 7)

TAPS = []  # (t, k, ring, dy, dx); t = SBUF slot, k = DRAM plane
for i in range(K):
    for j in range(K):
        if i == 3 and j == 3:
            continue
        k = i * K + j
        t = k if k < 24 else k - 1
        TAPS.append((t, k, int(_INDEX[i, j]), 3 - i, 3 - j))


def _runs(lo, hi):
    """Maximal [lo,hi) sub-runs of taps sharing one attention ring."""
    runs = []
    for t, k, r, dy, dx in TAPS[lo:hi]:
        if runs and runs[-1][2] == r and runs[-1][1] == t:
            runs[-1][1] = t + 1
        else:
            runs.append([t, t + 1, r])
    return [tuple(x) for x in runs]


def _band_matrix() -> np.ndarray:
    band = np.zeros((128, BANDW), dtype=np.float32)
    for p in range(128):
        band[p, p + C0] = 1.0
    return band


def _build():
    nc = bacc.Bacc("TRN2", target_bir_lowering=False, debug=False,
                   num_devices=N_CORES)
    aff = nc.dram_tensor("affinity", [B_CORE, 49, 2, 128, W], FP32,
                         kind="ExternalInput").ap()
    att = nc.dram_tensor("attention", [B_CORE, 4, 2, 128, W], FP32,
                         kind="ExternalInput").ap()
    cs = nc.dram_tensor("current_segmentation", [B_CORE, 1, 2, 128, W], FP32,
                        kind="ExternalInput").ap()
    co = nc.dram_tensor("coarse_segmentation", [B_CORE, 1, 2, 128, W], FP32,
                        kind="ExternalInput").ap()
    band = nc.dram_tensor("band", [128, BANDW], BF16, kind="ExternalInput").ap()
    out = nc.dram_tensor("out", [B_CORE, 1, 2, 128, W], FP32,
                         kind="ExternalOutput").ap()

    with tile.TileContext(nc) as tc:
        with tc.tile_pool(name="const", bufs=1) as cpool, \
             tc.tile_pool(name="zp", bufs=2) as zpool, \
             tc.tile_pool(name="sp", bufs=3) as spool, \
             tc.tile_pool(name="az", bufs=2) as azpool, \
             tc.tile_pool(name="inp", bufs=2) as ipool, \
             tc.tile_pool(name="ep", bufs=1) as epool, \
             tc.tile_pool(name="ps", bufs=2, space="PSUM") as pspool:

            bandt = cpool.tile([128, BANDW], BF16)
            nc.scalar.dma_start(out=bandt[:], in_=band[:, :])
            ident = bandt[:, C0:C0 + 128]

            zts = [None] * B_CORE
            attfs = [None] * B_CORE
            csts = [None] * B_CORE
            cots = [None] * B_CORE
            att3rs = [None] * B_CORE

            def attf_ring(img, r):
                # per-ring-channel DMAs, staggered by need time: chunk 0's
                # z-mult gates only on the small ring-0 slice
                nc.scalar.dma_start(
                    out=attfs[img][:, r, :, :],
                    in_=att[img, r].transpose([1, 0, 2]))

            def prologue_early(img):
                zt = zpool.tile([128, NT, 2, WG], BF16, tag="zt")
                nc.gpsimd.memset(zt[:, :, :, 0:GW], 0)
                nc.gpsimd.memset(zt[:, :, :, GW + W:], 0)
                attf = ipool.tile([128, 4, 2, W], FP32, tag="attf")
                zts[img] = zt
                attfs[img] = attf
                attf_ring(img, 0)

            def prologue_late(img):
                # segmentation maps + the att3 close plane are first needed
                # by the epilogue/closes; keep them out of the early window
                cst = ipool.tile([128, 2, W], FP32, tag="cst")
                nc.gpsimd.dma_start(out=cst[:],
                                    in_=cs[img, 0].transpose([1, 0, 2]))
                cot = ipool.tile([128, 2, W], FP32, tag="cot")
                nc.gpsimd.dma_start(out=cot[:],
                                    in_=co[img, 0].transpose([1, 0, 2]))
                att3r = ipool.tile([128, 2, W], BF16, tag="att3r")
                nc.scalar.activation(att3r[:], attfs[img][:, 3, :, :],
                                     mybir.ActivationFunctionType.Copy,
                                     bias=EPS)
                csts[img] = cst
                cots[img] = cot
                att3rs[img] = att3r

            prologue_early(0)

            all_stgs = {0: [None] * len(CHUNKS), 1: [None] * len(CHUNKS)}

            def issue_chunk(img, ci):
                lo, hi = CHUNKS[ci]
                k_lo = lo if lo < 24 else lo + 1
                if img == 1 and ci in (0, 1):
                    # dedicated buffers: these issue mid-image-0 with no
                    # rotation WAR, so the sync ring never drains at the
                    # image boundary
                    stg = spool.tile([128, 4, 2, W], FP32, tag=f"stgd{ci}",
                                     bufs=1)
                else:
                    stg = spool.tile([128, STG_TAPS, 2, W], FP32, tag="stg")
                nc.sync.dma_start(
                    out=stg[:, 0:hi - lo, :, :],
                    in_=aff[img, k_lo:k_lo + hi - lo].transpose([2, 0, 1, 3]))
                all_stgs[img][ci] = stg

            for img in range(B_CORE):
                zt = zts[img]
                attf = attfs[img]

                psU = pspool.tile([128, 2, W], FP32, tag="U")
                psT = pspool.tile([128, 2, W], FP32, tag="T")
                psA = pspool.tile([128, 2, W], FP32, tag="A")
                first_pe_a = True

                # graded affinity chunk DMAs (sync/HWDGE ring only); image
                # 1's first two chunks are slotted into the FIFO right
                # after image 0's chunk 5 so later rotation-stalled issues
                # never leave the DMA pipe empty
                stgs = all_stgs[img]
                for ci in range(len(CHUNKS)):
                    if stgs[ci] is None:
                        issue_chunk(img, ci)
                    if img == 0 and ci == 5:
                        issue_chunk(1, 0)
                        issue_chunk(1, 1)
                if img == 0:
                    attf_ring(0, 1)

                for ci, (lo, hi) in enumerate(CHUNKS):
                    n = hi - lo
                    stg = stgs[ci]
                    # z = att_r * aff -> bf16 (DVE)
                    for rlo, rhi, ring in _runs(lo, hi):
                        nc.vector.tensor_tensor(
                            out=zt[:, rlo:rhi, :, GW:GW + W],
                            in0=stg[:, rlo - lo:rhi - lo, :, :],
                            in1=attf[:, ring:ring + 1, :, :].broadcast_to(
                                [128, rhi - rlo, 2, W]),
                            op=mybir.AluOpType.mult)
                    # U (PE, identity band)
                    for t in range(lo, hi):
                        nc.tensor.matmul(
                            out=psU[:], lhsT=ident,
                            rhs=zt[:, t, :, GW:GW + W],
                            start=(t == 0), stop=False)
                    # T (PE): row shift dy via band diagonal, col shift dx
                    # via guarded columns; halo rows read the OTHER block
                    for t, k, r, dy, dx in TAPS[lo:hi]:
                        nc.tensor.matmul(
                            out=psT[:],
                            lhsT=bandt[:, C0 + dy:C0 + dy + 128],
                            rhs=zt[:, t, :, GW + dx:GW + dx + W],
                            start=(t == 0), stop=False)
                    for t, k, r, dy, dx in TAPS[lo:hi]:
                        if dy > 0:
                            nc.tensor.matmul(
                                out=psT[:, 0, :],
                                lhsT=bandt[:, 3 + dy:3 + dy + 128],
                                rhs=zt[:, t, 1, GW + dx:GW + dx + W],
                                start=False, stop=False)
                        elif dy < 0:
                            nc.tensor.matmul(
                                out=psT[:, 1, :],
                                lhsT=bandt[:, 259 + dy:259 + dy + 128],
                                rhs=zt[:, t, 0, GW + dx:GW + dx + W],
                                start=False, stop=False)
                    # A: |z| (ACT, 4-tap ops) + PE matmuls
                    for s_lo in range(lo, hi, 4):
                        s_hi = min(s_lo + 4, hi)
                        azt = azpool.tile([128, 4, 2, W], BF16, tag="azt")
                        nc.scalar.activation(
                            azt[:, 0:s_hi - s_lo, :, :],
                            zt[:, s_lo:s_hi, :, GW:GW + W],
                            mybir.ActivationFunctionType.Abs)
                        for tt in range(s_hi - s_lo):
                            nc.tensor.matmul(
                                out=psA[:], lhsT=ident,
                                rhs=azt[:, tt, :, :],
                                start=first_pe_a, stop=False)
                            first_pe_a = False
                    # remaining inputs, staggered by first need so they
                    # never steal bandwidth from the next affinity chunk
                    if img == 0:
                        if ci == 0:
                            attf_ring(0, 2)
                        elif ci == 1:
                            attf_ring(0, 3)
                        elif ci == 2:
                            prologue_late(0)
                            prologue_early(1)
                        elif ci == 4:
                            attf_ring(1, 1)
                        elif ci == 5:
                            attf_ring(1, 2)
                            attf_ring(1, 3)
                        elif ci == 6:
                            prologue_late(1)

                # close the accumulation groups with the +att3(+eps) taps
                nc.tensor.matmul(out=psU[:], lhsT=ident, rhs=att3rs[img][:],
                                 start=False, stop=True)
                nc.tensor.matmul(out=psA[:], lhsT=ident, rhs=att3rs[img][:],
                                 start=False, stop=True)
                nc.tensor.matmul(out=psT[:], lhsT=ident, rhs=att3rs[img][:],
                                 start=False, stop=True)

                # ---- epilogue (DVE; PSUM readers). out = (T*cs - U*co)*r
                # + co re-associated as  m1*r - q  with q = (U*co)*r - co,
                # so q is finished while the PE still closes psT and only a
                # 3-op chain trails the last close.
                m2 = epool.tile([128, 2, W], FP32, tag="m2")
                nc.vector.tensor_mul(m2[:], psU[:], cots[img][:])
                rcp = epool.tile([128, 2, W], FP32, tag="rcp")
                nc.vector.reciprocal_approx_fast(out=rcp[:], in_=psA[:])
                nc.vector.tensor_mul(m2[:], m2[:], rcp[:])
                nc.vector.tensor_sub(m2[:], m2[:], cots[img][:])
                m1 = epool.tile([128, 2, W], FP32, tag="m1")
                nc.vector.tensor_mul(m1[:], psT[:], csts[img][:])
                nc.vector.tensor_mul(m1[:], m1[:], rcp[:])
                nc.vector.tensor_sub(m1[:], m1[:], m2[:])
                # image 1's store rides the lower-latency HWDGE ring (ACT
                # is idle by then); image 0's stays on SWDGE mid-stream
                out_eng = nc.gpsimd if img == 0 else nc.scalar
                out_eng.dma_start(out=out[img, 0].transpose([1, 0, 2]),
                                  in_=m1[:])

    nc.compile()
    return nc


_NC_CACHE = None


def _get_nc():
    global _NC_CACHE
    if _NC_CACHE is None:
        _NC_CACHE = _build()
    return _NC_CACHE


def run(inputs: dict, trace: bool = False):
    """Run on 8 NeuronCores; returns (out [16,1,256,256], BassKernelResults)."""
    aff = np.ascontiguousarray(np.asarray(inputs["affinity"], dtype=np.float32))
    att = np.ascontiguousarray(np.asarray(inputs["attention"], dtype=np.float32))
    cs = np.ascontiguousarray(
        np.asarray(inputs["current_segmentation"], dtype=np.float32))
    co = np.ascontiguousarray(
        np.asarray(inputs["coarse_segmentation"], dtype=np.float32))
    band = _band_matrix().astype(bfloat16)

    nc = _get_nc()
    in_maps = []
    for c in range(N_CORES):
        s = slice(c * B_CORE, (c + 1) * B_CORE)
        in_maps.append({
            "affinity": np.ascontiguousarray(aff[s]).reshape(
                B_CORE, 49, 2, 128, W),
            "attention": np.ascontiguousarray(att[s]).reshape(
                B_CORE, 4, 2, 128, W),
            "current_segmentation": np.ascontiguousarray(cs[s]).reshape(
                B_CORE, 1, 2, 128, W),
            "coarse_segmentation": np.ascontiguousarray(co[s]).reshape(
                B_CORE, 1, 2, 128, W),
            "band": band,
        })
    last_err = None
    for attempt in range(3):
        try:
            res = run_bass_kernel_spmd(nc, in_maps, list(range(N_CORES)),
                                       trace=trace)
            break
        except Exception as e:  # transient NRT_EXEC_UNIT_UNRECOVERABLE flakes
            last_err = e
            import time
            time.sleep(10)
    else:
        raise last_err
    full = np.concatenate(
        [res.results[c]["out"].reshape(B_CORE, 1, H, W)
         for c in range(N_CORES)], axis=0)
    return full, res


def kernel(**inputs) -> np.ndarray:
    out, _ = run(inputs, trace=False)
    return out


# revision 45
# speedup vs baseline: 1.0351x; 1.0351x over previous
"""DYSPN attention-conv kernel for Trainium2 (8 NeuronCores, batch-parallel).

Math (unfold/fold pair collapses algebraically; per image, tap k=(i,j) != center,
ring r = INDEX[i,j], dy = 3-i, dx = 3-j):
    z_k[y,x] = att_r[y,x] * aff_k[y,x]
    U[y,x]   = sum_k z_k[y,x]                       (S_ppt - att3)
    A[y,x]   = sum_k |z_k[y,x]|                     (S_prime - att3; att >= 0)
    T[y,x]   = sum_k z_k[y+dy, x+dx]  (in-image)    (fold7(z))
    out = r * ((T+att3)*cs - (U+att3)*co) + co,  r = 1/(A+att3+eps)

Design: 2 images/core; one bf16 z tile per image holding all 48 taps and BOTH
128-row blocks ([128, 48, 2, 264]) so cross-image double buffering fits in
SBUF and the T halo is chunk-local. fp32 affinity chunks DMA into a rotating
fp32 staging pool (sync/HWDGE ring, nothing else on it), DVE multiplies by
the ring-broadcast attention -> bf16 z. All reductions are PE bf16 matmuls
with 512-wide rhs covering both blocks: U/A via the identity band, T via
dy-shifted band windows plus per-tap halo matmuls. |z| on ACT (4-tap ops).
Guard zeroing via one-time Pool memsets. Other inputs ride the scalar HWDGE
+ gpsimd SWDGE rings; image 1's inputs are emitted behind image 0's early
ACT work so they don't steal bandwidth from chunk 0. Epilogue on DVE with
reciprocal_approx_fast; out store on the SWDGE ring.
"""
import sys

sys.path.insert(0, "/opt/trn_rl_repo")

import numpy as np
from ml_dtypes import bfloat16

import concourse.bass as bass  # noqa: F401  (registers engines)
import concourse.tile as tile
from concourse import bacc, mybir
from concourse.bass_utils import run_bass_kernel_spmd

FP32 = mybir.dt.float32
BF16 = mybir.dt.bfloat16

N_CORES = 8
B_FULL = 16
B_CORE = B_FULL // N_CORES  # 2 images per core
H = W = 256
K = 7
NT = 48                   # 49 taps minus center
GW = 4                    # zero guard columns each side of a z tap plane
WG = W + 2 * GW
BANDW = 390               # band[p, q] = 1 iff q == p + C0
C0 = 131
EPS = 1e-6
# tap chunks (t-order); edges never cross t=24 (the skipped center plane);
# graded small at both ends for a fast start and a short tail
CHUNKS = [(0, 4), (4, 8), (8, 12), (12, 16), (16, 20), (20, 24),
          (24, 28), (28, 32), (32, 36), (36, 40), (40, 44),
          (44, 46), (46, 48)]
STG_TAPS = 4              # staging tile capacity (max chunk size)

_INDEX = np.array([0, 0, 0, 0, 0, 0, 0,
                   0, 1, 1, 1, 1, 1, 0,
                   0, 1, 2, 2, 2, 1, 0,
                   0, 1, 2, 3, 2, 1, 0,
                   0, 1, 2, 2, 2, 1, 0,
                   0, 1, 1, 1, 1, 1, 0,
                   0, 0, 0, 0, 0, 0, 0], dtype=np.int64).reshape(7, 7)

TAPS = []  # (t, k, ring, dy, dx); t = SBUF slot, k = DRAM plane
for i in range(K):
    for j in range(K):
        if i == 3 and j == 3:
            continue
        k = i * K + j
        t = k if k < 24 else k - 1
        TAPS.append((t, k, int(_INDEX[i, j]), 3 - i, 3 - j))


def _runs(lo, hi):
    """Maximal [lo,hi) sub-runs of taps sharing one attention ring."""
    runs = []
    for t, k, r, dy, dx in TAPS[lo:hi]:
        if runs and runs[-1][2] == r and runs[-1][1] == t:
            runs[-1][1] = t + 1
        else:
            runs.append([t, t + 1, r])
    return [tuple(x) for x in runs]


def _band_matrix() -> np.ndarray:
    band = np.zeros((128, BANDW), dtype=np.float32)
    for p in range(128):
        band[p, p + C0] = 1.0
    return band


def _build():
    nc = bacc.Bacc("TRN2", target_bir_lowering=False, debug=False,
                   num_devices=N_CORES)
    aff = nc.dram_tensor("affinity", [B_CORE, 49, 2, 128, W], FP32,
                         kind="ExternalInput").ap()
    att = nc.dram_tensor("attention", [B_CORE, 4, 2, 128, W], FP32,
                         kind="ExternalInput").ap()
    cs = nc.dram_tensor("current_segmentation", [B_CORE, 1, 2, 128, W], FP32,
                        kind="ExternalInput").ap()
    co = nc.dram_tensor("coarse_segmentation", [B_CORE, 1, 2, 128, W], FP32,
                        kind="ExternalInput").ap()
    band = nc.dram_tensor("band", [128, BANDW], BF16, kind="ExternalInput").ap()
    out = nc.dram_tensor("out", [B_CORE, 1, 2, 128, W], FP32,
                         kind="ExternalOutput").ap()

    with tile.TileContext(nc) as tc:
        with tc.tile_pool(name="const", bufs=1) as cpool, \
             tc.tile_pool(name="zp", bufs=2) as zpool, \
             tc.tile_pool(name="sp", bufs=6) as spool, \
             tc.tile_pool(name="az", bufs=2) as azpool, \
             tc.tile_pool(name="inp", bufs=2) as ipool, \
             tc.tile_pool(name="ep", bufs=1) as epool, \
             tc.tile_pool(name="ps", bufs=2, space="PSUM") as pspool:

            bandt = cpool.tile([128, BANDW], BF16)
            nc.scalar.dma_start(out=bandt[:], in_=band[:, :])
            ident = bandt[:, C0:C0 + 128]

            zts = [None] * B_CORE
            attfs = [None] * B_CORE
            csts = [None] * B_CORE
            cots = [None] * B_CORE
            att3rs = [None] * B_CORE

            def attf_ring(img, r):
                # per-ring-channel DMAs, staggered by need time: chunk 0's
                # z-mult gates only on the small ring-0 slice
                nc.scalar.dma_start(
                    out=attfs[img][:, r, :, :],
                    in_=att[img, r].transpose([1, 0, 2]))

            def prologue_early(img):
                zt = zpool.tile([128, NT, 2, WG], BF16, tag="zt")
                nc.gpsimd.memset(zt[:, :, :, 0:GW], 0)
                nc.gpsimd.memset(zt[:, :, :, GW + W:], 0)
                attf = ipool.tile([128, 4, 2, W], FP32, tag="attf")
                zts[img] = zt
                attfs[img] = attf
                attf_ring(img, 0)

            def prologue_late(img):
                # segmentation maps + the att3 close plane are first needed
                # by the epilogue/closes; keep them out of the early window
                cst = ipool.tile([128, 2, W], FP32, tag="cst")
                nc.gpsimd.dma_start(out=cst[:],
                                    in_=cs[img, 0].transpose([1, 0, 2]))
                cot = ipool.tile([128, 2, W], FP32, tag="cot")
                nc.gpsimd.dma_start(out=cot[:],
                                    in_=co[img, 0].transpose([1, 0, 2]))
                att3r = ipool.tile([128, 2, W], BF16, tag="att3r")
                nc.scalar.activation(att3r[:], attfs[img][:, 3, :, :],
                                     mybir.ActivationFunctionType.Copy,
                                     bias=EPS)
                csts[img] = cst
                cots[img] = cot
                att3rs[img] = att3r

            prologue_early(0)

            for img in range(B_CORE):
                zt = zts[img]
                attf = attfs[img]

                psU = pspool.tile([128, 2, W], FP32, tag="U")
                psT = pspool.tile([128, 2, W], FP32, tag="T")
                psA = pspool.tile([128, 2, W], FP32, tag="A")
                first_pe_a = True

                # graded affinity chunk DMAs (sync/HWDGE ring only); image
                # 1's first chunk gets a dedicated staging buffer so its
                # issue never waits on the rotation at the image boundary
                stgs = []
                for ci, (lo, hi) in enumerate(CHUNKS):
                    k_lo = lo if lo < 24 else lo + 1
                    if img == 1 and ci == 0:
                        stg = spool.tile([128, 4, 2, W], FP32, tag="stg0",
                                         bufs=1)
                    else:
                        stg = spool.tile([128, STG_TAPS, 2, W], FP32,
                                         tag="stg")
                    nc.sync.dma_start(
                        out=stg[:, 0:hi - lo, :, :],
                        in_=aff[img, k_lo:k_lo + hi - lo].transpose(
                            [2, 0, 1, 3]))
                    stgs.append(stg)
                if img == 0:
                    attf_ring(0, 1)

                for ci, (lo, hi) in enumerate(CHUNKS):
                    n = hi - lo
                    stg = stgs[ci]
                    # z = att_r * aff -> bf16 (DVE)
                    for rlo, rhi, ring in _runs(lo, hi):
                        nc.vector.tensor_tensor(
                            out=zt[:, rlo:rhi, :, GW:GW + W],
                            in0=stg[:, rlo - lo:rhi - lo, :, :],
                            in1=attf[:, ring:ring + 1, :, :].broadcast_to(
                                [128, rhi - rlo, 2, W]),
                            op=mybir.AluOpType.mult)
                    # U (PE, identity band)
                    for t in range(lo, hi):
                        nc.tensor.matmul(
                            out=psU[:], lhsT=ident,
                            rhs=zt[:, t, :, GW:GW + W],
                            start=(t == 0), stop=False)
                    # T (PE): row shift dy via band diagonal, col shift dx
                    # via guarded columns; halo rows read the OTHER block
                    for t, k, r, dy, dx in TAPS[lo:hi]:
                        nc.tensor.matmul(
                            out=psT[:],
                            lhsT=bandt[:, C0 + dy:C0 + dy + 128],
                            rhs=zt[:, t, :, GW + dx:GW + dx + W],
                            start=(t == 0), stop=False)
                    for t, k, r, dy, dx in TAPS[lo:hi]:
                        if dy > 0:
                            nc.tensor.matmul(
                                out=psT[:, 0, :],
                                lhsT=bandt[:, 3 + dy:3 + dy + 128],
                                rhs=zt[:, t, 1, GW + dx:GW + dx + W],
                                start=False, stop=False)
                        elif dy < 0:
                            nc.tensor.matmul(
                                out=psT[:, 1, :],
                                lhsT=bandt[:, 259 + dy:259 + dy + 128],
                                rhs=zt[:, t, 0, GW + dx:GW + dx + W],
                                start=False, stop=False)
                    # A: |z| (ACT, 4-tap ops) + PE matmuls
                    for s_lo in range(lo, hi, 4):
                        s_hi = min(s_lo + 4, hi)
                        azt = azpool.tile([128, 4, 2, W], BF16, tag="azt")
                        nc.scalar.activation(
                            azt[:, 0:s_hi - s_lo, :, :],
                            zt[:, s_lo:s_hi, :, GW:GW + W],
                            mybir.ActivationFunctionType.Abs)
                        for tt in range(s_hi - s_lo):
                            nc.tensor.matmul(
                                out=psA[:], lhsT=ident,
                                rhs=azt[:, tt, :, :],
                                start=first_pe_a, stop=False)
                            first_pe_a = False
                    # remaining inputs, staggered by first need so they
                    # never steal bandwidth from the next affinity chunk
                    if img == 0:
                        if ci == 0:
                            attf_ring(0, 2)
                        elif ci == 1:
                            attf_ring(0, 3)
                        elif ci == 3:
                            prologue_late(0)
                            prologue_early(1)
                        elif ci == 7:
                            attf_ring(1, 1)
                        elif ci == 9:
                            attf_ring(1, 2)
                            attf_ring(1, 3)
                        elif ci == 10:
                            prologue_late(1)

                # close the accumulation groups with the +att3(+eps) taps
                nc.tensor.matmul(out=psU[:], lhsT=ident, rhs=att3rs[img][:],
                                 start=False, stop=True)
                nc.tensor.matmul(out=psA[:], lhsT=ident, rhs=att3rs[img][:],
                                 start=False, stop=True)
                nc.tensor.matmul(out=psT[:], lhsT=ident, rhs=att3rs[img][:],
                                 start=False, stop=True)

                # ---- epilogue (DVE; PSUM readers), ordered so only the
                # short m1 -> sub -> mul -> add chain trails the psT close
                m2 = epool.tile([128, 2, W], FP32, tag="m2")
                nc.vector.tensor_mul(m2[:], psU[:], cots[img][:])
                rcp = epool.tile([128, 2, W], FP32, tag="rcp")
                nc.vector.reciprocal_approx_fast(out=rcp[:], in_=psA[:])
                m1 = epool.tile([128, 2, W], FP32, tag="m1")
                nc.vector.tensor_mul(m1[:], psT[:], csts[img][:])
                nc.vector.tensor_sub(m1[:], m1[:], m2[:])
                nc.vector.tensor_mul(m1[:], m1[:], rcp[:])
                nc.vector.tensor_add(m1[:], m1[:], cots[img][:])
                nc.gpsimd.dma_start(out=out[img, 0].transpose([1, 0, 2]),
                                    in_=m1[:])

    nc.compile()
    return nc


_NC_CACHE = None


def _get_nc():
    global _NC_CACHE
    if _NC_CACHE is None:
        _NC_CACHE = _build()
    return _NC_CACHE


def run(inputs: dict, trace: bool = False):
    """Run on 8 NeuronCores; returns (out [16,1,256,256], BassKernelResults)."""
    aff = np.ascontiguousarray(np.asarray(inputs["affinity"], dtype=np.float32))
    att = np.ascontiguousarray(np.asarray(inputs["attention"], dtype=np.float32))
    cs = np.ascontiguousarray(
        np.asarray(inputs["current_segmentation"], dtype=np.float32))
    co = np.ascontiguousarray(
        np.asarray(inputs["coarse_segmentation"], dtype=np.float32))
    band = _band_matrix().astype(bfloat16)

    nc = _get_nc()
    in_maps = []
    for c in range(N_CORES):
        s = slice(c * B_CORE, (c + 1) * B_CORE)
        in_maps.append({
            "affinity": np.ascontiguousarray(aff[s]).reshape(
                B_CORE, 49, 2, 128, W),
            "attention": np.ascontiguousarray(att[s]).reshape(
                B_CORE, 4, 2, 128, W),
            "current_segmentation": np.ascontiguousarray(cs[s]).reshape(
                B_CORE, 1, 2, 128, W),
            "coarse_segmentation": np.ascontiguousarray(co[s]).reshape(
                B_CORE, 1, 2, 128, W),
            "band": band,
        })
    last_err = None
    for attempt in range(3):
        try:
            res = run_bass_kernel_spmd(nc, in_maps, list(range(N_CORES)),
                                       trace=trace)
            break
        except Exception as e:  # transient NRT_EXEC_UNIT_UNRECOVERABLE flakes
            last_err = e
            import time
            time.sleep(10)
    else:
        raise last_err
    full = np.concatenate(
        [res.results[c]["out"].reshape(B_CORE, 1, H, W)
         for c in range(N_CORES)], axis=0)
    return full, res


def kernel(**inputs) -> np.ndarray:
    out, _ = run(inputs, trace=False)
    return out


# revision 46
# speedup vs baseline: 1.0589x; 1.0230x over previous
"""DYSPN attention-conv kernel for Trainium2 (8 NeuronCores, batch-parallel).

Math (unfold/fold pair collapses algebraically; per image, tap k=(i,j) != center,
ring r = INDEX[i,j], dy = 3-i, dx = 3-j):
    z_k[y,x] = att_r[y,x] * aff_k[y,x]
    U[y,x]   = sum_k z_k[y,x]                       (S_ppt - att3)
    A[y,x]   = sum_k |z_k[y,x]|                     (S_prime - att3; att >= 0)
    T[y,x]   = sum_k z_k[y+dy, x+dx]  (in-image)    (fold7(z))
    out = r * ((T+att3)*cs - (U+att3)*co) + co,  r = 1/(A+att3+eps)

Design: 2 images/core; one bf16 z tile per image holding all 48 taps and BOTH
128-row blocks ([128, 48, 2, 264]) so cross-image double buffering fits in
SBUF and the T halo is chunk-local. fp32 affinity chunks DMA into a rotating
fp32 staging pool (sync/HWDGE ring, nothing else on it), DVE multiplies by
the ring-broadcast attention -> bf16 z. All reductions are PE bf16 matmuls
with 512-wide rhs covering both blocks: U/A via the identity band, T via
dy-shifted band windows plus per-tap halo matmuls. |z| on ACT (4-tap ops).
Guard zeroing via one-time Pool memsets. Other inputs ride the scalar HWDGE
+ gpsimd SWDGE rings; image 1's inputs are emitted behind image 0's early
ACT work so they don't steal bandwidth from chunk 0. Epilogue on DVE with
reciprocal_approx_fast; out store on the SWDGE ring.
"""
import sys

sys.path.insert(0, "/opt/trn_rl_repo")

import numpy as np
from ml_dtypes import bfloat16

import concourse.bass as bass  # noqa: F401  (registers engines)
import concourse.tile as tile
from concourse import bacc, mybir
from concourse.bass_utils import run_bass_kernel_spmd

FP32 = mybir.dt.float32
BF16 = mybir.dt.bfloat16

N_CORES = 8
B_FULL = 16
B_CORE = B_FULL // N_CORES  # 2 images per core
H = W = 256
K = 7
NT = 48                   # 49 taps minus center
GW = 4                    # zero guard columns each side of a z tap plane
WG = W + 2 * GW
BANDW = 390               # band[p, q] = 1 iff q == p + C0
C0 = 131
EPS = 1e-6
# tap chunks (t-order); edges never cross t=24 (the skipped center plane);
# graded small at both ends for a fast start and a short tail
CHUNKS = [(0, 4), (4, 8), (8, 12), (12, 16), (16, 20), (20, 24),
          (24, 28), (28, 32), (32, 36), (36, 40), (40, 44),
          (44, 46), (46, 48)]
STG_TAPS = 4              # staging tile capacity (max chunk size)

_INDEX = np.array([0, 0, 0, 0, 0, 0, 0,
                   0, 1, 1, 1, 1, 1, 0,
                   0, 1, 2, 2, 2, 1, 0,
                   0, 1, 2, 3, 2, 1, 0,
                   0, 1, 2, 2, 2, 1, 0,
                   0, 1, 1, 1, 1, 1, 0,
                   0, 0, 0, 0, 0, 0, 0], dtype=np.int64).reshape(7, 7)

TAPS = []  # (t, k, ring, dy, dx); t = SBUF slot, k = DRAM plane
for i in range(K):
    for j in range(K):
        if i == 3 and j == 3:
            continue
        k = i * K + j
        t = k if k < 24 else k - 1
        TAPS.append((t, k, int(_INDEX[i, j]), 3 - i, 3 - j))


def _runs(lo, hi):
    """Maximal [lo,hi) sub-runs of taps sharing one attention ring."""
    runs = []
    for t, k, r, dy, dx in TAPS[lo:hi]:
        if runs and runs[-1][2] == r and runs[-1][1] == t:
            runs[-1][1] = t + 1
        else:
            runs.append([t, t + 1, r])
    return [tuple(x) for x in runs]


def _band_matrix() -> np.ndarray:
    band = np.zeros((128, BANDW), dtype=np.float32)
    for p in range(128):
        band[p, p + C0] = 1.0
    return band


def _build():
    nc = bacc.Bacc("TRN2", target_bir_lowering=False, debug=False,
                   num_devices=N_CORES)
    aff = nc.dram_tensor("affinity", [B_CORE, 49, 2, 128, W], FP32,
                         kind="ExternalInput").ap()
    att = nc.dram_tensor("attention", [B_CORE, 4, 2, 128, W], FP32,
                         kind="ExternalInput").ap()
    cs = nc.dram_tensor("current_segmentation", [B_CORE, 1, 2, 128, W], FP32,
                        kind="ExternalInput").ap()
    co = nc.dram_tensor("coarse_segmentation", [B_CORE, 1, 2, 128, W], FP32,
                        kind="ExternalInput").ap()
    band = nc.dram_tensor("band", [128, BANDW], BF16, kind="ExternalInput").ap()
    out = nc.dram_tensor("out", [B_CORE, 1, 2, 128, W], FP32,
                         kind="ExternalOutput").ap()

    with tile.TileContext(nc) as tc:
        with tc.tile_pool(name="const", bufs=1) as cpool, \
             tc.tile_pool(name="zp", bufs=2) as zpool, \
             tc.tile_pool(name="sp", bufs=6) as spool, \
             tc.tile_pool(name="az", bufs=2) as azpool, \
             tc.tile_pool(name="inp", bufs=2) as ipool, \
             tc.tile_pool(name="ep", bufs=1) as epool, \
             tc.tile_pool(name="ps", bufs=2, space="PSUM") as pspool:

            bandt = cpool.tile([128, BANDW], BF16)
            nc.scalar.dma_start(out=bandt[:], in_=band[:, :])
            ident = bandt[:, C0:C0 + 128]

            zts = [None] * B_CORE
            attfs = [None] * B_CORE
            csts = [None] * B_CORE
            cots = [None] * B_CORE
            att3rs = [None] * B_CORE

            def attf_ring(img, r):
                # per-ring-channel DMAs, staggered by need time: chunk 0's
                # z-mult gates only on the small ring-0 slice
                nc.scalar.dma_start(
                    out=attfs[img][:, r, :, :],
                    in_=att[img, r].transpose([1, 0, 2]))

            def prologue_early(img):
                zt = zpool.tile([128, NT, 2, WG], BF16, tag="zt")
                nc.gpsimd.memset(zt[:, :, :, 0:GW], 0)
                nc.gpsimd.memset(zt[:, :, :, GW + W:], 0)
                attf = ipool.tile([128, 4, 2, W], FP32, tag="attf")
                zts[img] = zt
                attfs[img] = attf
                attf_ring(img, 0)

            def prologue_late(img):
                # segmentation maps + the att3 close plane are first needed
                # by the epilogue/closes; keep them out of the early window
                cst = ipool.tile([128, 2, W], FP32, tag="cst")
                nc.gpsimd.dma_start(out=cst[:],
                                    in_=cs[img, 0].transpose([1, 0, 2]))
                cot = ipool.tile([128, 2, W], FP32, tag="cot")
                nc.gpsimd.dma_start(out=cot[:],
                                    in_=co[img, 0].transpose([1, 0, 2]))
                att3r = ipool.tile([128, 2, W], BF16, tag="att3r")
                nc.scalar.activation(att3r[:], attfs[img][:, 3, :, :],
                                     mybir.ActivationFunctionType.Copy,
                                     bias=EPS)
                csts[img] = cst
                cots[img] = cot
                att3rs[img] = att3r

            prologue_early(0)

            for img in range(B_CORE):
                zt = zts[img]
                attf = attfs[img]

                psU = pspool.tile([128, 2, W], FP32, tag="U")
                psT = pspool.tile([128, 2, W], FP32, tag="T")
                psA = pspool.tile([128, 2, W], FP32, tag="A")
                first_pe_a = True

                # graded affinity chunk DMAs (sync/HWDGE ring only); image
                # 1's first chunk gets a dedicated staging buffer so its
                # issue never waits on the rotation at the image boundary
                stgs = []
                for ci, (lo, hi) in enumerate(CHUNKS):
                    k_lo = lo if lo < 24 else lo + 1
                    if img == 1 and ci == 0:
                        stg = spool.tile([128, 4, 2, W], FP32, tag="stg0",
                                         bufs=1)
                    else:
                        stg = spool.tile([128, STG_TAPS, 2, W], FP32,
                                         tag="stg")
                    nc.sync.dma_start(
                        out=stg[:, 0:hi - lo, :, :],
                        in_=aff[img, k_lo:k_lo + hi - lo].transpose(
                            [2, 0, 1, 3]))
                    stgs.append(stg)
                if img == 0:
                    attf_ring(0, 1)

                for ci, (lo, hi) in enumerate(CHUNKS):
                    n = hi - lo
                    stg = stgs[ci]
                    # z = att_r * aff -> bf16 (DVE); runs capped at 2 taps
                    # so the PE starts ~1us after each chunk lands
                    for rlo, rhi, ring in _runs(lo, hi):
                        for s in range(rlo, rhi, 2):
                            e = min(s + 2, rhi)
                            nc.vector.tensor_tensor(
                                out=zt[:, s:e, :, GW:GW + W],
                                in0=stg[:, s - lo:e - lo, :, :],
                                in1=attf[:, ring:ring + 1, :, :].broadcast_to(
                                    [128, e - s, 2, W]),
                                op=mybir.AluOpType.mult)
                    # U (PE, identity band)
                    for t in range(lo, hi):
                        nc.tensor.matmul(
                            out=psU[:], lhsT=ident,
                            rhs=zt[:, t, :, GW:GW + W],
                            start=(t == 0), stop=False)
                    # T (PE): row shift dy via band diagonal, col shift dx
                    # via guarded columns; halo rows read the OTHER block
                    for t, k, r, dy, dx in TAPS[lo:hi]:
                        nc.tensor.matmul(
                            out=psT[:],
                            lhsT=bandt[:, C0 + dy:C0 + dy + 128],
                            rhs=zt[:, t, :, GW + dx:GW + dx + W],
                            start=(t == 0), stop=False)
                    for t, k, r, dy, dx in TAPS[lo:hi]:
                        if dy > 0:
                            nc.tensor.matmul(
                                out=psT[:, 0, :],
                                lhsT=bandt[:, 3 + dy:3 + dy + 128],
                                rhs=zt[:, t, 1, GW + dx:GW + dx + W],
                                start=False, stop=False)
                        elif dy < 0:
                            nc.tensor.matmul(
                                out=psT[:, 1, :],
                                lhsT=bandt[:, 259 + dy:259 + dy + 128],
                                rhs=zt[:, t, 0, GW + dx:GW + dx + W],
                                start=False, stop=False)
                    # A: |z| (ACT, 4-tap ops) + PE matmuls
                    for s_lo in range(lo, hi, 4):
                        s_hi = min(s_lo + 4, hi)
                        azt = azpool.tile([128, 4, 2, W], BF16, tag="azt")
                        nc.scalar.activation(
                            azt[:, 0:s_hi - s_lo, :, :],
                            zt[:, s_lo:s_hi, :, GW:GW + W],
                            mybir.ActivationFunctionType.Abs)
                        for tt in range(s_hi - s_lo):
                            nc.tensor.matmul(
                                out=psA[:], lhsT=ident,
                                rhs=azt[:, tt, :, :],
                                start=first_pe_a, stop=False)
                            first_pe_a = False
                    # remaining inputs, staggered by first need so they
                    # never steal bandwidth from the next affinity chunk
                    if img == 0:
                        if ci == 0:
                            attf_ring(0, 2)
                        elif ci == 1:
                            attf_ring(0, 3)
                        elif ci == 3:
                            prologue_late(0)
                            prologue_early(1)
                        elif ci == 7:
                            attf_ring(1, 1)
                        elif ci == 9:
                            attf_ring(1, 2)
                            attf_ring(1, 3)
                        elif ci == 10:
                            prologue_late(1)

                # close the accumulation groups with the +att3(+eps) taps
                nc.tensor.matmul(out=psU[:], lhsT=ident, rhs=att3rs[img][:],
                                 start=False, stop=True)
                nc.tensor.matmul(out=psA[:], lhsT=ident, rhs=att3rs[img][:],
                                 start=False, stop=True)
                nc.tensor.matmul(out=psT[:], lhsT=ident, rhs=att3rs[img][:],
                                 start=False, stop=True)

                # ---- epilogue (DVE; PSUM readers), ordered so only the
                # short m1 -> sub -> mul -> add chain trails the psT close
                m2 = epool.tile([128, 2, W], FP32, tag="m2")
                nc.vector.tensor_mul(m2[:], psU[:], cots[img][:])
                rcp = epool.tile([128, 2, W], FP32, tag="rcp")
                nc.vector.reciprocal_approx_fast(out=rcp[:], in_=psA[:])
                m1 = epool.tile([128, 2, W], FP32, tag="m1")
                nc.vector.tensor_mul(m1[:], psT[:], csts[img][:])
                nc.vector.tensor_sub(m1[:], m1[:], m2[:])
                nc.vector.tensor_mul(m1[:], m1[:], rcp[:])
                nc.vector.tensor_add(m1[:], m1[:], cots[img][:])
                nc.gpsimd.dma_start(out=out[img, 0].transpose([1, 0, 2]),
                                    in_=m1[:])

    nc.compile()
    return nc


_NC_CACHE = None


def _get_nc():
    global _NC_CACHE
    if _NC_CACHE is None:
        _NC_CACHE = _build()
    return _NC_CACHE


def run(inputs: dict, trace: bool = False):
    """Run on 8 NeuronCores; returns (out [16,1,256,256], BassKernelResults)."""
    aff = np.ascontiguousarray(np.asarray(inputs["affinity"], dtype=np.float32))
    att = np.ascontiguousarray(np.asarray(inputs["attention"], dtype=np.float32))
    cs = np.ascontiguousarray(
        np.asarray(inputs["current_segmentation"], dtype=np.float32))
    co = np.ascontiguousarray(
        np.asarray(inputs["coarse_segmentation"], dtype=np.float32))
    band = _band_matrix().astype(bfloat16)

    nc = _get_nc()
    in_maps = []
    for c in range(N_CORES):
        s = slice(c * B_CORE, (c + 1) * B_CORE)
        in_maps.append({
            "affinity": np.ascontiguousarray(aff[s]).reshape(
                B_CORE, 49, 2, 128, W),
            "attention": np.ascontiguousarray(att[s]).reshape(
                B_CORE, 4, 2, 128, W),
            "current_segmentation": np.ascontiguousarray(cs[s]).reshape(
                B_CORE, 1, 2, 128, W),
            "coarse_segmentation": np.ascontiguousarray(co[s]).reshape(
                B_CORE, 1, 2, 128, W),
            "band": band,
        })
    last_err = None
    for attempt in range(3):
        try:
            res = run_bass_kernel_spmd(nc, in_maps, list(range(N_CORES)),
                                       trace=trace)
            break
        except Exception as e:  # transient NRT_EXEC_UNIT_UNRECOVERABLE flakes
            last_err = e
            import time
            time.sleep(10)
    else:
        raise last_err
    full = np.concatenate(
        [res.results[c]["out"].reshape(B_CORE, 1, H, W)
         for c in range(N_CORES)], axis=0)
    return full, res


def kernel(**inputs) -> np.ndarray:
    out, _ = run(inputs, trace=False)
    return out
